# revision 34
# baseline (speedup 1.0000x reference)
"""GPT2 attention (B=4, S=2048, D=1024, H=16) on 8 trn2 cores.

Sharding: data-parallel over batch (4) x tensor-parallel over heads (2 groups
of 8). Core c handles batch c//2, head group c%2. Each core computes its
partial output projection (row-split c_proj); the host sums the two partials
per batch and adds the (host-folded) biases. The host also pre-transposes
each core's activation slice (x^T, feature-major) -- pure layout marshalling,
like the weight slicing -- so the device never transposes x.

Per-core kernel, all matmuls in bf16 (fp32 PSUM accumulate):
  B: x^T arrives f32 feature-major, cast to bf16 in-flight by SWDGE DMA ->
     QKV projections. q^T,k^T stay feature-major and SBUF-resident (bias
     added on the vector engine); v is natural [s, head, d] with a ones
     column appended (softmax row sums).
  C: causal attention per 512-wide sq chunk j, head-pair p: transposed
     scores scoresT[sk,sq] = kT.T @ qT, p^T = exp(scoresT/8) (bf16, no
     max-subtract; |scores|/8 is small), diagonal blocks masked by an
     upper-triangular 0/1 multiply. PV accumulates NATURALLY:
     attn[sq, d(+sum)] += pT_block.T @ v_block, so the softmax row sums land
     per-partition and the reciprocal+normalize are cheap vector ops. The
     normalized chunk goes back to attnT layout via xbar DMA-transpose for
     the projection.
  D: out_partial = attnT.T @ c_proj_w (row slice).

To keep the PE busy (HAM clock-gate: >3.4us idle drops PE to 1.2GHz), the
QKV work of chunk j+1 and the projections of chunks 0..2 are emitted as
filler thunks paced evenly through the attention i-loops, covering the PE
bubbles that the ACT-engine exp latency would otherwise create.
"""
import sys

sys.path.insert(0, "/opt/trn_rl_repo")

from collections import deque
from contextlib import ExitStack

import numpy as np

import concourse.bass as bass
import concourse.bacc as bacc
import concourse.mybir as mybir
import concourse.tile as tile
from concourse.masks import make_upper_triangular
from concourse.bass_utils import run_bass_kernel_spmd

F32 = mybir.dt.float32
BF16 = mybir.dt.bfloat16
AF = mybir.ActivationFunctionType
OP = mybir.AluOpType

B, S, D, H = 4, 2048, 1024, 16
DH = 64            # head dim
NCORES = 8
GH = 8             # heads per core
GD = GH * DH       # 512 feature cols per core
ST = S // 128      # 16 s-tiles
KB = D // 128      # 8 contraction blocks
NJ = S // 512      # 4 sq chunks
MT = GD // 128     # 4 m-tiles (= head pairs)
LAG = 4            # PV emission lag behind scores (covers ACT exp latency)


def build_module():
    nc = bacc.Bacc(None, target_bir_lowering=False, debug=False)

    xt = nc.declare_dram_parameter("xt", [D, S], F32, isOutput=False)
    wq = nc.declare_dram_parameter("wq", [D, GD], F32, isOutput=False)
    wk = nc.declare_dram_parameter("wk", [D, GD], F32, isOutput=False)
    wv = nc.declare_dram_parameter("wv", [D, GD], F32, isOutput=False)
    wp = nc.declare_dram_parameter("wp", [GD, D], F32, isOutput=False)
    bqk = nc.declare_dram_parameter("bqk", [128, 2 * MT], F32, isOutput=False)
    out = nc.declare_dram_parameter("out", [S, D], F32, isOutput=True)

    with tile.TileContext(nc) as tc:
        _build_body(nc, tc, xt, wq, wk, wv, wp, bqk, out)
    nc.compile()
    return nc


def _build_body(nc, tc, xt, wq, wk, wv, wp, bqk, out):
    with ExitStack() as ctx:
        const = ctx.enter_context(tc.tile_pool(name="const", bufs=1))
        wpool = ctx.enter_context(tc.tile_pool(name="wpool", bufs=1))
        wppool = ctx.enter_context(tc.tile_pool(name="wppool", bufs=1))
        resid = ctx.enter_context(tc.tile_pool(name="resid", bufs=1))
        xin = ctx.enter_context(tc.tile_pool(name="xin", bufs=1))
        pTp = ctx.enter_context(tc.tile_pool(name="pTp", bufs=6))
        rcp = ctx.enter_context(tc.tile_pool(name="rcp", bufs=2))
        atm = ctx.enter_context(tc.tile_pool(name="atm", bufs=2))
        ost = ctx.enter_context(tc.tile_pool(name="ost", bufs=2))
        # PSUM budget (8 banks / 16KB): scores 8KB, PV accum 4KB, shared f32
        # matmul accumulator (QKV + proj) 2x2KB.
        psh = ctx.enter_context(tc.tile_pool(name="psh", bufs=2, space="PSUM"))
        psc = ctx.enter_context(tc.tile_pool(name="psc", bufs=2, space="PSUM"))
        pat = ctx.enter_context(tc.tile_pool(name="pat", bufs=1, space="PSUM"))

        # ---- constants FIRST: they use gpsimd (iota/memset), which must
        # not queue behind the SWDGE DMA-prep stream below.
        tri_f = const.tile([128, 128], F32)  # 1 where col >= row else 0
        make_upper_triangular(nc, tri_f[:], val=1.0, diag=True)
        tri = const.tile([128, 128], BF16)
        nc.vector.tensor_copy(tri[:], tri_f[:])
        ones_v = const.tile([128, ST * GH], F32)
        nc.gpsimd.memset(ones_v[:], 1.0)
        bqk_sb = const.tile([128, 2 * MT], F32)
        nc.sync.dma_start(bqk_sb[:], bqk.ap())
        # warm the ACT exp table while the PE is still in phase B
        dume = const.tile([128, 1], BF16)
        nc.scalar.activation(dume[:], bqk_sb[:, 0:1], AF.Exp, scale=0.125)

        # PE warmup: the HAM clock gate starts at 1.2GHz and only ramps to
        # 2.4GHz after ~3.4us of sustained activity. Dummy matmuls during
        # the initial DMA wait trip the gate so the real phase-B matmuls
        # run at full clock from the start.
        wrm = pat.tile([128, 2, 4, 128], F32, name="at")
        for _ in range(96):
            nc.tensor.matmul(wrm[:, 0, 0, :], lhsT=tri[:], rhs=tri[:],
                             start=True, stop=True)

        # ---- x^T residents [128 d, S], bf16. The j=0 column chunk comes
        # via fast HWDGE f32 loads + vector casts (the PE start depends on
        # it); the rest streams through SWDGE cast-DMA, interleaved with
        # the weights in consumption order.
        xt_sb = [resid.tile([128, S], BF16, name=f"xt{k}") for k in range(KB)]
        xf32 = []
        for k in range(KB):
            xf = xin.tile([128, 512], F32, name=f"xf{k}")
            nc.sync.dma_start(xf[:], xt.ap()[k * 128:(k + 1) * 128, 0:512])
            xf32.append(xf)
        for k in range(KB):
            nc.vector.tensor_copy(xt_sb[k][:, 0:512], xf32[k][:])

        wq_sb = [wpool.tile([128, GD], BF16, name=f"wq{k}") for k in range(KB)]
        wk_sb = [wpool.tile([128, GD], BF16, name=f"wk{k}") for k in range(KB)]
        wv_sb = [wpool.tile([128, GD], BF16, name=f"wv{k}") for k in range(KB)]
        wp_sb = [wppool.tile([128, 512], BF16, name=f"wp{i}") for i in range(8)]
        for k in range(KB):
            nc.gpsimd.dma_start(wq_sb[k][:], wq.ap()[k * 128:(k + 1) * 128, :])
        for j in range(1, NJ):
            for k in range(KB):
                nc.gpsimd.dma_start(
                    xt_sb[k][:, j * 512:(j + 1) * 512],
                    xt.ap()[k * 128:(k + 1) * 128, j * 512:(j + 1) * 512])
            wsb, wdr = ((wk_sb, wk), (wv_sb, wv), (wp_sb, wp))[j - 1]
            if j < 3:
                for k in range(KB):
                    nc.gpsimd.dma_start(
                        wsb[k][:], wdr.ap()[k * 128:(k + 1) * 128, :])
            else:
                for k4 in range(4):
                    for n in range(2):
                        nc.gpsimd.dma_start(
                            wp_sb[k4 * 2 + n][:],
                            wp.ap()[k4 * 128:(k4 + 1) * 128,
                                    n * 512:(n + 1) * 512])

        # ---- residents ----
        kT_sb = [resid.tile([128, S], BF16, name=f"kT{m}") for m in range(MT)]
        qT_sb = [resid.tile([128, S], BF16, name=f"qT{m}") for m in range(MT)]
        # v natural with ones column: [128 s-in-block, block i, head, 65]
        v_sb = resid.tile([128, ST, GH, DH + 1], BF16)
        nc.vector.tensor_copy(
            v_sb[:, :, :, DH],
            ones_v[:].rearrange("p (a b) -> p a b", a=ST))
        # attnT for the projection: [128 = (hh,d) of pair, pair, sq]
        aT_sb = resid.tile([128, MT, S], BF16)

        # ================= phase B thunk builder =================
        def build_B(j, borrow=False):
            """QKV thunks for chunk j, split by deadline: q thunks must
            complete before C(j) starts (scores rhs); k/v thunks are only
            consumed at C(j)'s diagonal iterations, so they can drain
            inside C(j)'s early i-loop, keeping the PE ahead of the ACT
            exp stream. With borrow=True thunks use the then-idle scores
            pool so consecutive thunks ping-pong instead of serializing."""

            def acc_tile(brw):
                if brw:
                    return psc.tile([128, 2, 512], F32, name="sc")[:, 0, :]
                return psh.tile([128, 512], F32, name="ps")

            def qk_thunk(m, which, brw=False):
                wsb, dst, bcol = (
                    (wq_sb, qT_sb, m) if which == 0 else (wk_sb, kT_sb, MT + m))
                ps = acc_tile(brw)
                for k in range(KB):
                    nc.tensor.matmul(
                        ps[:], lhsT=wsb[k][:, m * 128:(m + 1) * 128],
                        rhs=xt_sb[k][:, j * 512:(j + 1) * 512],
                        start=(k == 0), stop=(k == KB - 1))
                nc.vector.tensor_scalar_add(
                    dst[m][:, j * 512:(j + 1) * 512], ps[:],
                    bqk_sb[:, bcol:bcol + 1])

            def v_thunk(st_i, brw=False):
                i_blk = 4 * j + st_i
                ps = acc_tile(brw)
                for k in range(KB):
                    nc.tensor.matmul(
                        ps[:],
                        lhsT=xt_sb[k][:, i_blk * 128:(i_blk + 1) * 128],
                        rhs=wv_sb[k][:], start=(k == 0), stop=(k == KB - 1))
                nc.vector.tensor_copy(
                    v_sb[:, i_blk, :, 0:DH],
                    ps[:].rearrange("p (h d) -> p h d", h=GH))

            q_thunks = [lambda m=m: qk_thunk(m, 0, borrow and m == 0)
                        for m in range(MT)]
            kv_thunks = ([lambda m=m: qk_thunk(m, 1, borrow and m == 0)
                          for m in range(MT)] +
                         [lambda s=s: v_thunk(s, borrow) for s in range(4)])
            return q_thunks, kv_thunks

        # ================= phase D thunk builder =================
        def build_proj(j, tail=False):
            """Projection of sq chunk j: 8 thunks of 4 matmuls each.
            The tail chunk (after C is done) borrows the then-idle scores
            pool so consecutive thunks ping-pong instead of serializing."""
            thunks = []

            def proj_thunk(mi, n):
                if tail:
                    ps = psc.tile([128, 2, 512], F32, name="sc")[:, 0, :]
                else:
                    ps = psh.tile([128, 512], F32, name="ps")
                for k4 in range(4):
                    nc.tensor.matmul(
                        ps[:],
                        lhsT=aT_sb[:, k4, mi * 128:(mi + 1) * 128],
                        rhs=wp_sb[k4 * 2 + n][:],
                        start=(k4 == 0), stop=(k4 == 3))
                o_sb = ost.tile([128, 512], F32, name="o_sb")
                nc.vector.tensor_copy(o_sb[:], ps[:])
                nc.sync.dma_start(
                    out.ap()[mi * 128:(mi + 1) * 128,
                             n * 512:(n + 1) * 512], o_sb[:])
            for mi4 in range(4):
                for n in range(2):
                    thunks.append(
                        lambda mi=4 * j + mi4, n=n: proj_thunk(mi, n))
            return thunks

        # ================= emission =================
        fillers = deque()
        proj_deferred = []

        # B(0): emit only what C(0) p=0 needs inline (pair-0 q+k, all v);
        # the other pairs' q/k become fillers, drained during C(0)'s early
        # iterations before their pair comes up.
        b0q, b0kv = build_B(0, borrow=True)
        b0q[0]()
        b0kv[0]()
        for t in b0kv[4:8]:
            t()
        for m in range(1, MT):
            fillers.append(b0q[m])
            fillers.append(b0kv[m])

        for j in range(NJ):
            if j + 1 < NJ:
                qn, kvn = build_B(j + 1)
                fillers.extend(qn + kvn)
            else:
                fillers.extend(proj_deferred)
                proj_deferred = []

            iters = MT * (4 * j + 4)
            quota = len(fillers)
            drained = it = 0

            for p in range(MT):
                # natural PV accumulator (2 banks): cols [b*128, b*128+65)
                # of block b for head hh hold [attn | rowsum]
                at_ps = pat.tile([128, 2, 4, 128], F32, name="at")
                pv_pend = deque()

                def emit_pv(i, pT, b0_, at_ps=at_ps):
                    # start=True clears has_written for the WHOLE PSUM bank
                    # (hh selects the bank here), so only the first matmul
                    # into each bank may set it; later regions rely on the
                    # cleared bits to overwrite-then-accumulate.
                    for hh in range(2):
                        for b in range(b0_, 4):
                            nc.tensor.matmul(
                                at_ps[:, hh, b, 0:DH + 1],
                                lhsT=pT[:, hh, b * 128:(b + 1) * 128],
                                rhs=v_sb[:, i, 2 * p + hh, :],
                                start=(i == 0 and b == 0),
                                stop=(i == 4 * j + 3 and b == 3))

                for i in range(4 * j + 4):
                    c0 = max(0, i * 128 - j * 512)
                    sc = psc.tile([128, 2, 512], F32, name="sc")
                    for hh in range(2):
                        nc.tensor.matmul(
                            sc[:, hh, c0:],
                            lhsT=kT_sb[p][hh * 64:(hh + 1) * 64,
                                          i * 128:(i + 1) * 128],
                            rhs=qT_sb[p][hh * 64:(hh + 1) * 64,
                                         j * 512 + c0:(j + 1) * 512],
                            start=True, stop=True,
                            tile_position=(hh * 64, 0))
                    pT = pTp.tile([128, 2, 512], BF16, name="pT")
                    nc.scalar.activation(pT[:, :, c0:], sc[:, :, c0:],
                                         AF.Exp, scale=0.125)
                    if i * 128 >= j * 512:  # diagonal block: causal mask
                        nc.vector.tensor_tensor(
                            pT[:, :, c0:c0 + 128],
                            pT[:, :, c0:c0 + 128],
                            tri[:, None, :].broadcast_to([128, 2, 128]),
                            op=OP.mult)
                    pv_pend.append((i, pT, c0 // 128))
                    if len(pv_pend) > LAG:
                        emit_pv(*pv_pend.popleft())
                    it += 1
                    # pace against iters+8 so a few thunks remain to cover
                    # the PV flushes at the chunk's end
                    target = (it * quota) // (iters + 8)
                    while drained < target and fillers:
                        fillers.popleft()()
                        drained += 1
                while pv_pend:
                    emit_pv(*pv_pend.popleft())
                    # keep PE fed while the tail exps drain on ACT
                    if fillers and drained < quota:
                        fillers.popleft()()
                        drained += 1

                # normalize: rowsums sit at col 64 of each block, per
                # partition -> cheap reciprocal + broadcast multiply
                a_tmp = atm.tile([128, 4, 128], BF16, name="a_tmp")
                rc = rcp.tile([128, 2, 4, 1], F32, name="rc")
                nc.vector.reciprocal(rc[:], at_ps[:, :, :, DH:DH + 1])
                nc.vector.tensor_tensor(
                    a_tmp[:].rearrange("p b (hh d) -> p hh b d", hh=2),
                    at_ps[:, :, :, 0:DH],
                    rc[:].broadcast_to([128, 2, 4, DH]), op=OP.mult)
                # back to attnT layout for the projection via xbar DMA
                nc.sync.dma_start_transpose(
                    aT_sb[:, p, j * 512:(j + 1) * 512].rearrange(
                        "p (b s) -> p b s", b=4),
                    a_tmp[:].rearrange("p b s -> p (b s)"))

            while fillers:     # B(j+1) must be emitted before C(j+1)
                fillers.popleft()()
            if j < NJ - 1:
                proj_deferred.extend(build_proj(j))

        for t in build_proj(NJ - 1, tail=True):
            t()


_NC = None


def _get_module():
    global _NC
    if _NC is None:
        _NC = build_module()
    return _NC


def make_in_maps(hidden_states, c_attn_w, c_attn_b, c_proj_w):
    in_maps = []
    for c in range(NCORES):
        b, g = c // 2, c % 2
        cols = slice(g * GD, (g + 1) * GD)
        bq = np.ascontiguousarray(
            c_attn_b[g * GD:(g + 1) * GD].reshape(MT, 128).T)
        bk = np.ascontiguousarray(
            c_attn_b[D + g * GD:D + (g + 1) * GD].reshape(MT, 128).T)
        in_maps.append({
            "xt": np.ascontiguousarray(hidden_states[b].T),
            "wq": np.ascontiguousarray(c_attn_w[:, cols]),
            "wk": np.ascontiguousarray(c_attn_w[:, D + g * GD:D + (g + 1) * GD]),
            "wv": np.ascontiguousarray(
                c_attn_w[:, 2 * D + g * GD:2 * D + (g + 1) * GD]),
            "wp": np.ascontiguousarray(c_proj_w[g * GD:(g + 1) * GD, :]),
            "bqk": np.concatenate([bq, bk], axis=1).astype(np.float32),
        })
    return in_maps


def kernel(hidden_states, c_attn_w, c_attn_b, c_proj_w, c_proj_b, _trace=False):
    hidden_states = np.asarray(hidden_states, dtype=np.float32)
    c_attn_w = np.asarray(c_attn_w, dtype=np.float32)
    c_attn_b = np.asarray(c_attn_b, dtype=np.float32)
    c_proj_w = np.asarray(c_proj_w, dtype=np.float32)
    c_proj_b = np.asarray(c_proj_b, dtype=np.float32)

    nc = _get_module()
    in_maps = make_in_maps(hidden_states, c_attn_w, c_attn_b, c_proj_w)
    res = run_bass_kernel_spmd(nc, in_maps, list(range(NCORES)), trace=_trace)

    # v-bias is folded here: attn rows sum to 1, so +b_v passes through the
    # attention average and lands as b_v @ c_proj_w on the output.
    bias_eff = c_proj_b + c_attn_b[2 * D:3 * D] @ c_proj_w
    outp = np.empty((B, S, D), dtype=np.float32)
    for b in range(B):
        outp[b] = (res.results[2 * b]["out"] + res.results[2 * b + 1]["out"]
                   + bias_eff[None, :])
    if _trace:
        return outp, res
    return outp


# revision 35
# speedup vs baseline: 1.1293x; 1.1293x over previous
"""GPT2 attention (B=4, S=2048, D=1024, H=16) on 8 trn2 cores.

Sharding: data-parallel over batch (4) x tensor-parallel over heads (2 groups
of 8). Core c handles batch c//2, head group c%2. Each core computes its
partial output projection (row-split c_proj); the host sums the two partials
per batch and adds the (host-folded) biases. The host also pre-transposes
each core's activation slice (x^T, feature-major) -- pure layout marshalling,
like the weight slicing -- so the device never transposes x.

Per-core kernel, all matmuls in bf16 (fp32 PSUM accumulate):
  B: x^T arrives f32 feature-major, cast to bf16 in-flight by SWDGE DMA ->
     QKV projections. q^T,k^T stay feature-major and SBUF-resident (bias
     added on the vector engine); v is natural [s, head, d] with a ones
     column appended (softmax row sums).
  C: causal attention per 512-wide sq chunk j, head-pair p: transposed
     scores scoresT[sk,sq] = kT.T @ qT, p^T = exp(scoresT/8) (bf16, no
     max-subtract; |scores|/8 is small), diagonal blocks masked by an
     upper-triangular 0/1 multiply. PV accumulates NATURALLY:
     attn[sq, d(+sum)] += pT_block.T @ v_block, so the softmax row sums land
     per-partition and the reciprocal+normalize are cheap vector ops. The
     normalized chunk goes back to attnT layout via xbar DMA-transpose for
     the projection.
  D: out_partial = attnT.T @ c_proj_w (row slice).

To keep the PE busy (HAM clock-gate: >3.4us idle drops PE to 1.2GHz), the
QKV work of chunk j+1 and the projections of chunks 0..2 are emitted as
filler thunks paced evenly through the attention i-loops, covering the PE
bubbles that the ACT-engine exp latency would otherwise create.
"""
import sys

sys.path.insert(0, "/opt/trn_rl_repo")

from collections import deque
from contextlib import ExitStack

import numpy as np

import concourse.bass as bass
import concourse.bacc as bacc
import concourse.mybir as mybir
import concourse.tile as tile
from concourse.masks import make_upper_triangular
from concourse.bass_utils import run_bass_kernel_spmd

F32 = mybir.dt.float32
BF16 = mybir.dt.bfloat16
AF = mybir.ActivationFunctionType
OP = mybir.AluOpType

B, S, D, H = 4, 2048, 1024, 16
DH = 64            # head dim
NCORES = 8
GH = 8             # heads per core
GD = GH * DH       # 512 feature cols per core
ST = S // 128      # 16 s-tiles
KB = D // 128      # 8 contraction blocks
NJ = S // 512      # 4 sq chunks
MT = GD // 128     # 4 m-tiles (= head pairs)
LAG = 4            # PV emission lag behind scores (covers ACT exp latency)


def build_module():
    nc = bacc.Bacc(None, target_bir_lowering=False, debug=False)

    xt = nc.declare_dram_parameter("xt", [D, S], F32, isOutput=False)
    wq = nc.declare_dram_parameter("wq", [D, GD], F32, isOutput=False)
    wk = nc.declare_dram_parameter("wk", [D, GD], F32, isOutput=False)
    wv = nc.declare_dram_parameter("wv", [D, GD], F32, isOutput=False)
    wp = nc.declare_dram_parameter("wp", [GD, D], F32, isOutput=False)
    bqk = nc.declare_dram_parameter("bqk", [128, 2 * MT], F32, isOutput=False)
    out = nc.declare_dram_parameter("out", [S, D], F32, isOutput=True)

    with tile.TileContext(nc) as tc:
        _build_body(nc, tc, xt, wq, wk, wv, wp, bqk, out)
    nc.compile()
    return nc


def _build_body(nc, tc, xt, wq, wk, wv, wp, bqk, out):
    with ExitStack() as ctx:
        const = ctx.enter_context(tc.tile_pool(name="const", bufs=1))
        wpool = ctx.enter_context(tc.tile_pool(name="wpool", bufs=1))
        wppool = ctx.enter_context(tc.tile_pool(name="wppool", bufs=1))
        resid = ctx.enter_context(tc.tile_pool(name="resid", bufs=1))
        xin = ctx.enter_context(tc.tile_pool(name="xin", bufs=1))
        pTp = ctx.enter_context(tc.tile_pool(name="pTp", bufs=6))
        rcp = ctx.enter_context(tc.tile_pool(name="rcp", bufs=2))
        atm = ctx.enter_context(tc.tile_pool(name="atm", bufs=2))
        ost = ctx.enter_context(tc.tile_pool(name="ost", bufs=2))
        # PSUM budget (8 banks / 16KB): scores 8KB, PV accum 4KB, shared f32
        # matmul accumulator (QKV + proj) 2x2KB.
        psh = ctx.enter_context(tc.tile_pool(name="psh", bufs=2, space="PSUM"))
        psc = ctx.enter_context(tc.tile_pool(name="psc", bufs=2, space="PSUM"))
        pat = ctx.enter_context(tc.tile_pool(name="pat", bufs=1, space="PSUM"))

        # ---- constants FIRST: they use gpsimd (iota/memset), which must
        # not queue behind the SWDGE DMA-prep stream below.
        tri_f = const.tile([128, 128], F32)  # 1 where col >= row else 0
        make_upper_triangular(nc, tri_f[:], val=1.0, diag=True)
        tri = const.tile([128, 128], BF16)
        nc.vector.tensor_copy(tri[:], tri_f[:])
        ones_v = const.tile([128, ST * GH], F32)
        nc.gpsimd.memset(ones_v[:], 1.0)
        bqk_sb = const.tile([128, 2 * MT], F32)
        nc.sync.dma_start(bqk_sb[:], bqk.ap())
        # warm the ACT exp table while the PE is still in phase B
        dume = const.tile([128, 1], BF16)
        nc.scalar.activation(dume[:], bqk_sb[:, 0:1], AF.Exp, scale=0.125)



        # ---- x^T residents [128 d, S], bf16. The j=0 column chunk comes
        # via fast HWDGE f32 loads + vector casts (the PE start depends on
        # it); the rest streams through SWDGE cast-DMA, interleaved with
        # the weights in consumption order.
        xt_sb = [resid.tile([128, S], BF16, name=f"xt{k}") for k in range(KB)]
        xf32 = []
        for k in range(KB):
            xf = xin.tile([128, 512], F32, name=f"xf{k}")
            nc.sync.dma_start(xf[:], xt.ap()[k * 128:(k + 1) * 128, 0:512])
            xf32.append(xf)
        for k in range(KB):
            nc.vector.tensor_copy(xt_sb[k][:, 0:512], xf32[k][:])

        wq_sb = [wpool.tile([128, GD], BF16, name=f"wq{k}") for k in range(KB)]
        wk_sb = [wpool.tile([128, GD], BF16, name=f"wk{k}") for k in range(KB)]
        wv_sb = [wpool.tile([128, GD], BF16, name=f"wv{k}") for k in range(KB)]
        wp_sb = [wppool.tile([128, 512], BF16, name=f"wp{i}") for i in range(8)]
        for k in range(KB):
            nc.gpsimd.dma_start(wq_sb[k][:], wq.ap()[k * 128:(k + 1) * 128, :])
        for j in range(1, NJ):
            for k in range(KB):
                nc.gpsimd.dma_start(
                    xt_sb[k][:, j * 512:(j + 1) * 512],
                    xt.ap()[k * 128:(k + 1) * 128, j * 512:(j + 1) * 512])
            wsb, wdr = ((wk_sb, wk), (wv_sb, wv), (wp_sb, wp))[j - 1]
            if j < 3:
                for k in range(KB):
                    nc.gpsimd.dma_start(
                        wsb[k][:], wdr.ap()[k * 128:(k + 1) * 128, :])
            else:
                for k4 in range(4):
                    for n in range(2):
                        nc.gpsimd.dma_start(
                            wp_sb[k4 * 2 + n][:],
                            wp.ap()[k4 * 128:(k4 + 1) * 128,
                                    n * 512:(n + 1) * 512])

        # ---- residents ----
        kT_sb = [resid.tile([128, S], BF16, name=f"kT{m}") for m in range(MT)]
        qT_sb = [resid.tile([128, S], BF16, name=f"qT{m}") for m in range(MT)]
        # v natural with ones column: [128 s-in-block, block i, head, 65]
        v_sb = resid.tile([128, ST, GH, DH + 1], BF16)
        nc.vector.tensor_copy(
            v_sb[:, :, :, DH],
            ones_v[:].rearrange("p (a b) -> p a b", a=ST))
        # attnT for the projection: [128 = (hh,d) of pair, pair, sq]
        aT_sb = resid.tile([128, MT, S], BF16)

        # ================= phase B thunk builder =================
        def build_B(j, borrow=False):
            """QKV thunks for chunk j, split by deadline: q thunks must
            complete before C(j) starts (scores rhs); k/v thunks are only
            consumed at C(j)'s diagonal iterations, so they can drain
            inside C(j)'s early i-loop, keeping the PE ahead of the ACT
            exp stream. With borrow=True thunks use the then-idle scores
            pool so consecutive thunks ping-pong instead of serializing."""

            def acc_tile(brw):
                if brw:
                    return psc.tile([128, 2, 512], F32, name="sc")[:, 0, :]
                return psh.tile([128, 512], F32, name="ps")

            def qk_thunk(m, which, brw=False):
                wsb, dst, bcol = (
                    (wq_sb, qT_sb, m) if which == 0 else (wk_sb, kT_sb, MT + m))
                ps = acc_tile(brw)
                for k in range(KB):
                    nc.tensor.matmul(
                        ps[:], lhsT=wsb[k][:, m * 128:(m + 1) * 128],
                        rhs=xt_sb[k][:, j * 512:(j + 1) * 512],
                        start=(k == 0), stop=(k == KB - 1))
                nc.vector.tensor_scalar_add(
                    dst[m][:, j * 512:(j + 1) * 512], ps[:],
                    bqk_sb[:, bcol:bcol + 1])

            def v_thunk(st_i, brw=False):
                i_blk = 4 * j + st_i
                ps = acc_tile(brw)
                for k in range(KB):
                    nc.tensor.matmul(
                        ps[:],
                        lhsT=xt_sb[k][:, i_blk * 128:(i_blk + 1) * 128],
                        rhs=wv_sb[k][:], start=(k == 0), stop=(k == KB - 1))
                nc.vector.tensor_copy(
                    v_sb[:, i_blk, :, 0:DH],
                    ps[:].rearrange("p (h d) -> p h d", h=GH))

            q_thunks = [lambda m=m: qk_thunk(m, 0, borrow and m == 0)
                        for m in range(MT)]
            kv_thunks = ([lambda m=m: qk_thunk(m, 1, borrow and m == 0)
                          for m in range(MT)] +
                         [lambda s=s: v_thunk(s, borrow) for s in range(4)])
            return q_thunks, kv_thunks

        # ================= phase D thunk builder =================
        def build_proj(j, tail=False):
            """Projection of sq chunk j: 8 thunks of 4 matmuls each.
            The tail chunk (after C is done) borrows the then-idle scores
            pool so consecutive thunks ping-pong instead of serializing."""
            thunks = []

            def proj_thunk(mi, n):
                if tail:
                    ps = psc.tile([128, 2, 512], F32, name="sc")[:, 0, :]
                else:
                    ps = psh.tile([128, 512], F32, name="ps")
                for k4 in range(4):
                    nc.tensor.matmul(
                        ps[:],
                        lhsT=aT_sb[:, k4, mi * 128:(mi + 1) * 128],
                        rhs=wp_sb[k4 * 2 + n][:],
                        start=(k4 == 0), stop=(k4 == 3))
                o_sb = ost.tile([128, 512], F32, name="o_sb")
                nc.vector.tensor_copy(o_sb[:], ps[:])
                nc.sync.dma_start(
                    out.ap()[mi * 128:(mi + 1) * 128,
                             n * 512:(n + 1) * 512], o_sb[:])
            for mi4 in range(4):
                for n in range(2):
                    thunks.append(
                        lambda mi=4 * j + mi4, n=n: proj_thunk(mi, n))
            return thunks

        # ================= emission =================
        fillers = deque()
        proj_deferred = []

        # B(0): emit only what C(0) p=0 needs inline (pair-0 q+k, all v);
        # the other pairs' q/k become fillers, drained during C(0)'s early
        # iterations before their pair comes up.
        b0q, b0kv = build_B(0, borrow=True)
        b0q[0]()
        b0kv[0]()
        for t in b0kv[4:8]:
            t()
        for m in range(1, MT):
            fillers.append(b0q[m])
            fillers.append(b0kv[m])

        for j in range(NJ):
            if j + 1 < NJ:
                qn, kvn = build_B(j + 1)
                fillers.extend(qn + kvn)
            else:
                fillers.extend(proj_deferred)
                proj_deferred = []

            iters = MT * (4 * j + 4)
            quota = len(fillers)
            drained = it = 0

            for p in range(MT):
                # natural PV accumulator (2 banks): cols [b*128, b*128+65)
                # of block b for head hh hold [attn | rowsum]
                at_ps = pat.tile([128, 2, 4, 128], F32, name="at")
                pv_pend = deque()

                def emit_pv(i, pT, b0_, at_ps=at_ps):
                    # start=True clears has_written for the WHOLE PSUM bank
                    # (hh selects the bank here), so only the first matmul
                    # into each bank may set it; later regions rely on the
                    # cleared bits to overwrite-then-accumulate.
                    for hh in range(2):
                        for b in range(b0_, 4):
                            nc.tensor.matmul(
                                at_ps[:, hh, b, 0:DH + 1],
                                lhsT=pT[:, hh, b * 128:(b + 1) * 128],
                                rhs=v_sb[:, i, 2 * p + hh, :],
                                start=(i == 0 and b == 0),
                                stop=(i == 4 * j + 3 and b == 3))

                for i in range(4 * j + 4):
                    c0 = max(0, i * 128 - j * 512)
                    sc = psc.tile([128, 2, 512], F32, name="sc")
                    for hh in range(2):
                        nc.tensor.matmul(
                            sc[:, hh, c0:],
                            lhsT=kT_sb[p][hh * 64:(hh + 1) * 64,
                                          i * 128:(i + 1) * 128],
                            rhs=qT_sb[p][hh * 64:(hh + 1) * 64,
                                         j * 512 + c0:(j + 1) * 512],
                            start=True, stop=True,
                            tile_position=(hh * 64, 0))
                    pT = pTp.tile([128, 2, 512], BF16, name="pT")
                    nc.scalar.activation(pT[:, :, c0:], sc[:, :, c0:],
                                         AF.Exp, scale=0.125)
                    if i * 128 >= j * 512:  # diagonal block: causal mask
                        nc.vector.tensor_tensor(
                            pT[:, :, c0:c0 + 128],
                            pT[:, :, c0:c0 + 128],
                            tri[:, None, :].broadcast_to([128, 2, 128]),
                            op=OP.mult)
                    pv_pend.append((i, pT, c0 // 128))
                    if len(pv_pend) > LAG:
                        emit_pv(*pv_pend.popleft())
                    it += 1
                    # pace against iters+8 so a few thunks remain to cover
                    # the PV flushes at the chunk's end
                    target = (it * quota) // (iters + 8)
                    while drained < target and fillers:
                        fillers.popleft()()
                        drained += 1
                while pv_pend:
                    emit_pv(*pv_pend.popleft())
                    # keep PE fed while the tail exps drain on ACT
                    if fillers and drained < quota:
                        fillers.popleft()()
                        drained += 1

                # normalize: rowsums sit at col 64 of each block, per
                # partition -> cheap reciprocal + broadcast multiply
                a_tmp = atm.tile([128, 4, 128], BF16, name="a_tmp")
                rc = rcp.tile([128, 2, 4, 1], F32, name="rc")
                nc.vector.reciprocal(rc[:], at_ps[:, :, :, DH:DH + 1])
                nc.vector.tensor_tensor(
                    a_tmp[:].rearrange("p b (hh d) -> p hh b d", hh=2),
                    at_ps[:, :, :, 0:DH],
                    rc[:].broadcast_to([128, 2, 4, DH]), op=OP.mult)
                # back to attnT layout for the projection via xbar DMA
                nc.sync.dma_start_transpose(
                    aT_sb[:, p, j * 512:(j + 1) * 512].rearrange(
                        "p (b s) -> p b s", b=4),
                    a_tmp[:].rearrange("p b s -> p (b s)"))

            while fillers:     # B(j+1) must be emitted before C(j+1)
                fillers.popleft()()
            if j < NJ - 1:
                proj_deferred.extend(build_proj(j))

        for t in build_proj(NJ - 1, tail=True):
            t()


_NC = None


def _get_module():
    global _NC
    if _NC is None:
        _NC = build_module()
    return _NC


def make_in_maps(hidden_states, c_attn_w, c_attn_b, c_proj_w):
    in_maps = []
    for c in range(NCORES):
        b, g = c // 2, c % 2
        cols = slice(g * GD, (g + 1) * GD)
        bq = np.ascontiguousarray(
            c_attn_b[g * GD:(g + 1) * GD].reshape(MT, 128).T)
        bk = np.ascontiguousarray(
            c_attn_b[D + g * GD:D + (g + 1) * GD].reshape(MT, 128).T)
        in_maps.append({
            "xt": np.ascontiguousarray(hidden_states[b].T),
            "wq": np.ascontiguousarray(c_attn_w[:, cols]),
            "wk": np.ascontiguousarray(c_attn_w[:, D + g * GD:D + (g + 1) * GD]),
            "wv": np.ascontiguousarray(
                c_attn_w[:, 2 * D + g * GD:2 * D + (g + 1) * GD]),
            "wp": np.ascontiguousarray(c_proj_w[g * GD:(g + 1) * GD, :]),
            "bqk": np.concatenate([bq, bk], axis=1).astype(np.float32),
        })
    return in_maps


def kernel(hidden_states, c_attn_w, c_attn_b, c_proj_w, c_proj_b, _trace=False):
    hidden_states = np.asarray(hidden_states, dtype=np.float32)
    c_attn_w = np.asarray(c_attn_w, dtype=np.float32)
    c_attn_b = np.asarray(c_attn_b, dtype=np.float32)
    c_proj_w = np.asarray(c_proj_w, dtype=np.float32)
    c_proj_b = np.asarray(c_proj_b, dtype=np.float32)

    nc = _get_module()
    in_maps = make_in_maps(hidden_states, c_attn_w, c_attn_b, c_proj_w)
    res = run_bass_kernel_spmd(nc, in_maps, list(range(NCORES)), trace=_trace)

    # v-bias is folded here: attn rows sum to 1, so +b_v passes through the
    # attention average and lands as b_v @ c_proj_w on the output.
    bias_eff = c_proj_b + c_attn_b[2 * D:3 * D] @ c_proj_w
    outp = np.empty((B, S, D), dtype=np.float32)
    for b in range(B):
        outp[b] = (res.results[2 * b]["out"] + res.results[2 * b + 1]["out"]
                   + bias_eff[None, :])
    if _trace:
        return outp, res
    return outp


# revision 39
# speedup vs baseline: 1.1453x; 1.0141x over previous
"""GPT2 attention (B=4, S=2048, D=1024, H=16) on 8 trn2 cores.

Sharding: data-parallel over batch (4) x tensor-parallel over heads (2 groups
of 8). Core c handles batch c//2, head group c%2. Each core computes its
partial output projection (row-split c_proj); the host sums the two partials
per batch and adds the (host-folded) biases. The host also pre-transposes
each core's activation slice (x^T, feature-major) -- pure layout marshalling,
like the weight slicing -- so the device never transposes x.

Per-core kernel, all matmuls in bf16 (fp32 PSUM accumulate):
  B: x^T arrives f32 feature-major, cast to bf16 in-flight by SWDGE DMA ->
     QKV projections. q^T,k^T stay feature-major and SBUF-resident (bias
     added on the vector engine); v is natural [s, head, d] with a ones
     column appended (softmax row sums).
  C: causal attention per 512-wide sq chunk j, head-pair p: transposed
     scores scoresT[sk,sq] = kT.T @ qT, p^T = exp(scoresT/8) (bf16, no
     max-subtract; |scores|/8 is small), diagonal blocks masked by an
     upper-triangular 0/1 multiply. PV accumulates NATURALLY:
     attn[sq, d(+sum)] += pT_block.T @ v_block, so the softmax row sums land
     per-partition and the reciprocal+normalize are cheap vector ops. The
     normalized chunk goes back to attnT layout via xbar DMA-transpose for
     the projection.
  D: out_partial = attnT.T @ c_proj_w (row slice).

To keep the PE busy (HAM clock-gate: >3.4us idle drops PE to 1.2GHz), the
QKV work of chunk j+1 and the projections of chunks 0..2 are emitted as
filler thunks paced evenly through the attention i-loops, covering the PE
bubbles that the ACT-engine exp latency would otherwise create.
"""
import sys

sys.path.insert(0, "/opt/trn_rl_repo")

from collections import deque
from contextlib import ExitStack

import numpy as np

import concourse.bass as bass
import concourse.bacc as bacc
import concourse.mybir as mybir
import concourse.tile as tile
from concourse.masks import make_upper_triangular
from concourse.bass_utils import run_bass_kernel_spmd

F32 = mybir.dt.float32
BF16 = mybir.dt.bfloat16
AF = mybir.ActivationFunctionType
OP = mybir.AluOpType

B, S, D, H = 4, 2048, 1024, 16
DH = 64            # head dim
NCORES = 8
GH = 8             # heads per core
GD = GH * DH       # 512 feature cols per core
ST = S // 128      # 16 s-tiles
KB = D // 128      # 8 contraction blocks
NJ = S // 512      # 4 sq chunks
MT = GD // 128     # 4 m-tiles (= head pairs)
LAG = 4            # PV emission lag behind scores (covers ACT exp latency)


def build_module():
    nc = bacc.Bacc(None, target_bir_lowering=False, debug=False)

    xt = nc.declare_dram_parameter("xt", [D, S], F32, isOutput=False)
    wq = nc.declare_dram_parameter("wq", [D, GD], F32, isOutput=False)
    wk = nc.declare_dram_parameter("wk", [D, GD], F32, isOutput=False)
    wv = nc.declare_dram_parameter("wv", [D, GD], F32, isOutput=False)
    wp = nc.declare_dram_parameter("wp", [GD, D], F32, isOutput=False)
    bqk = nc.declare_dram_parameter("bqk", [128, 2 * MT], F32, isOutput=False)
    out = nc.declare_dram_parameter("out", [S, D], F32, isOutput=True)

    with tile.TileContext(nc) as tc:
        _build_body(nc, tc, xt, wq, wk, wv, wp, bqk, out)
    nc.compile()
    return nc


def _build_body(nc, tc, xt, wq, wk, wv, wp, bqk, out):
    with ExitStack() as ctx:
        const = ctx.enter_context(tc.tile_pool(name="const", bufs=1))
        wpool = ctx.enter_context(tc.tile_pool(name="wpool", bufs=1))
        wppool = ctx.enter_context(tc.tile_pool(name="wppool", bufs=1))
        resid = ctx.enter_context(tc.tile_pool(name="resid", bufs=1))
        xin = ctx.enter_context(tc.tile_pool(name="xin", bufs=1))
        pTp = ctx.enter_context(tc.tile_pool(name="pTp", bufs=6))
        rcp = ctx.enter_context(tc.tile_pool(name="rcp", bufs=2))
        atm = ctx.enter_context(tc.tile_pool(name="atm", bufs=2))
        ost = ctx.enter_context(tc.tile_pool(name="ost", bufs=2))
        # PSUM budget (8 banks / 16KB): scores 8KB, PV accum 4KB, shared f32
        # matmul accumulator (QKV + proj) 2x2KB.
        psh = ctx.enter_context(tc.tile_pool(name="psh", bufs=2, space="PSUM"))
        psc = ctx.enter_context(tc.tile_pool(name="psc", bufs=2, space="PSUM"))
        pat = ctx.enter_context(tc.tile_pool(name="pat", bufs=1, space="PSUM"))

        # ---- constants FIRST: they use gpsimd (iota/memset), which must
        # not queue behind the SWDGE DMA-prep stream below.
        tri_f = const.tile([128, 128], F32)  # 1 where col >= row else 0
        make_upper_triangular(nc, tri_f[:], val=1.0, diag=True)
        tri = const.tile([128, 128], BF16)
        nc.vector.tensor_copy(tri[:], tri_f[:])
        ones_v = const.tile([128, ST * GH], F32)
        nc.gpsimd.memset(ones_v[:], 1.0)
        bqk_sb = const.tile([128, 2 * MT], F32)
        nc.sync.dma_start(bqk_sb[:], bqk.ap())
        # warm the ACT exp table while the PE is still in phase B
        dume = const.tile([128, 1], BF16)
        nc.scalar.activation(dume[:], bqk_sb[:, 0:1], AF.Exp, scale=0.125)



        # ---- x^T residents [128 d, S], bf16. The j=0 column chunk comes
        # via fast HWDGE f32 loads + vector casts (the PE start depends on
        # it); the rest streams through SWDGE cast-DMA, interleaved with
        # the weights in consumption order.
        xt_sb = [resid.tile([128, S], BF16, name=f"xt{k}") for k in range(KB)]
        xf32 = []
        for k in range(KB):
            xf = xin.tile([128, 512], F32, name=f"xf{k}")
            nc.sync.dma_start(xf[:], xt.ap()[k * 128:(k + 1) * 128, 0:512])
            xf32.append(xf)
        for k in range(KB):
            nc.vector.tensor_copy(xt_sb[k][:, 0:512], xf32[k][:])

        wq_sb = [wpool.tile([128, GD], BF16, name=f"wq{k}") for k in range(KB)]
        wk_sb = [wpool.tile([128, GD], BF16, name=f"wk{k}") for k in range(KB)]
        wv_sb = [wpool.tile([128, GD], BF16, name=f"wv{k}") for k in range(KB)]
        wp_sb = [wppool.tile([128, 512], BF16, name=f"wp{i}") for i in range(8)]
        for k in range(KB):
            nc.gpsimd.dma_start(wq_sb[k][:], wq.ap()[k * 128:(k + 1) * 128, :])
        for j in range(1, NJ):
            for k in range(KB):
                nc.gpsimd.dma_start(
                    xt_sb[k][:, j * 512:(j + 1) * 512],
                    xt.ap()[k * 128:(k + 1) * 128, j * 512:(j + 1) * 512])
            wsb, wdr = ((wk_sb, wk), (wv_sb, wv), (wp_sb, wp))[j - 1]
            if j < 3:
                for k in range(KB):
                    nc.gpsimd.dma_start(
                        wsb[k][:], wdr.ap()[k * 128:(k + 1) * 128, :])
            else:
                for k4 in range(4):
                    for n in range(2):
                        nc.gpsimd.dma_start(
                            wp_sb[k4 * 2 + n][:],
                            wp.ap()[k4 * 128:(k4 + 1) * 128,
                                    n * 512:(n + 1) * 512])

        # ---- residents ----
        kT_sb = [resid.tile([128, S], BF16, name=f"kT{m}") for m in range(MT)]
        qT_sb = [resid.tile([128, S], BF16, name=f"qT{m}") for m in range(MT)]
        # v natural with ones column: [128 s-in-block, block i, head, 65]
        v_sb = resid.tile([128, ST, GH, DH + 1], BF16)
        nc.vector.tensor_copy(
            v_sb[:, :, :, DH],
            ones_v[:].rearrange("p (a b) -> p a b", a=ST))
        # attnT for the projection: [128 = (hh,d) of pair, pair, sq]
        aT_sb = resid.tile([128, MT, S], BF16)

        # ================= phase B thunk builder =================
        def build_B(j, borrow=False):
            """QKV thunks for chunk j, split by deadline: q thunks must
            complete before C(j) starts (scores rhs); k/v thunks are only
            consumed at C(j)'s diagonal iterations, so they can drain
            inside C(j)'s early i-loop, keeping the PE ahead of the ACT
            exp stream. With borrow=True thunks use the then-idle scores
            pool so consecutive thunks ping-pong instead of serializing."""

            def acc_tile(brw):
                if brw:
                    return psc.tile([128, 2, 512], F32, name="sc")[:, 0, :]
                return psh.tile([128, 512], F32, name="ps")

            def qk_thunk(m, which, brw=False):
                wsb, dst, bcol = (
                    (wq_sb, qT_sb, m) if which == 0 else (wk_sb, kT_sb, MT + m))
                ps = acc_tile(brw)
                for k in range(KB):
                    nc.tensor.matmul(
                        ps[:], lhsT=wsb[k][:, m * 128:(m + 1) * 128],
                        rhs=xt_sb[k][:, j * 512:(j + 1) * 512],
                        start=(k == 0), stop=(k == KB - 1))
                nc.vector.tensor_scalar_add(
                    dst[m][:, j * 512:(j + 1) * 512], ps[:],
                    bqk_sb[:, bcol:bcol + 1])

            def v_thunk(st_i, brw=False):
                i_blk = 4 * j + st_i
                ps = acc_tile(brw)
                for k in range(KB):
                    nc.tensor.matmul(
                        ps[:],
                        lhsT=xt_sb[k][:, i_blk * 128:(i_blk + 1) * 128],
                        rhs=wv_sb[k][:], start=(k == 0), stop=(k == KB - 1))
                nc.vector.tensor_copy(
                    v_sb[:, i_blk, :, 0:DH],
                    ps[:].rearrange("p (h d) -> p h d", h=GH))

            q_thunks = [lambda m=m: qk_thunk(m, 0, borrow and m == 0)
                        for m in range(MT)]
            kv_thunks = ([lambda m=m: qk_thunk(m, 1, borrow and m == 0)
                          for m in range(MT)] +
                         [lambda s=s: v_thunk(s, borrow) for s in range(4)])
            return q_thunks, kv_thunks

        # ================= phase D thunk builder =================
        def build_proj(j, tail=False):
            """Projection of sq chunk j: 8 thunks of 4 matmuls each.
            The tail chunk (after C is done) borrows the then-idle scores
            pool so consecutive thunks ping-pong instead of serializing."""
            thunks = []

            def proj_thunk(mi, n):
                if tail:
                    ps = psc.tile([128, 2, 512], F32, name="sc")[:, 0, :]
                else:
                    ps = psh.tile([128, 512], F32, name="ps")
                for k4 in range(4):
                    nc.tensor.matmul(
                        ps[:],
                        lhsT=aT_sb[:, k4, mi * 128:(mi + 1) * 128],
                        rhs=wp_sb[k4 * 2 + n][:],
                        start=(k4 == 0), stop=(k4 == 3))
                o_sb = ost.tile([128, 512], F32, name="o_sb")
                nc.vector.tensor_copy(o_sb[:], ps[:])
                nc.sync.dma_start(
                    out.ap()[mi * 128:(mi + 1) * 128,
                             n * 512:(n + 1) * 512], o_sb[:])
            for mi4 in range(4):
                for n in range(2):
                    thunks.append(
                        lambda mi=4 * j + mi4, n=n: proj_thunk(mi, n))
            return thunks

        # ================= emission =================
        fillers = deque()
        proj_deferred = []

        # B(0): emit only what C(0) p=0 needs inline (pair-0 q+k, all v);
        # the other pairs' q/k become fillers, drained during C(0)'s early
        # iterations before their pair comes up.
        b0q, b0kv = build_B(0, borrow=True)
        b0q[0]()
        b0kv[0]()
        for t in b0kv[4:8]:
            t()
        for m in range(1, MT):
            fillers.append(b0q[m])
            fillers.append(b0kv[m])

        for j in range(NJ):
            if j + 1 < NJ:
                qn, kvn = build_B(j + 1)
                for m in range(MT):      # interleave q,k per pair, then v
                    fillers.append(qn[m])
                    fillers.append(kvn[m])
                fillers.extend(kvn[4:])
            else:
                fillers.extend(proj_deferred)
                proj_deferred = []

            iters = MT * (4 * j + 4)
            quota = len(fillers)
            drained = it = 0

            for p in range(MT):
                # natural PV accumulator (2 banks): cols [b*128, b*128+65)
                # of block b for head hh hold [attn | rowsum]
                at_ps = pat.tile([128, 2, 4, 128], F32, name="at")
                pv_pend = deque()

                def emit_pv(i, pT, b0_, at_ps=at_ps):
                    # start=True clears has_written for the WHOLE PSUM bank
                    # (hh selects the bank here), so only the first matmul
                    # into each bank may set it; later regions rely on the
                    # cleared bits to overwrite-then-accumulate.
                    for hh in range(2):
                        for b in range(b0_, 4):
                            nc.tensor.matmul(
                                at_ps[:, hh, b, 0:DH + 1],
                                lhsT=pT[:, hh, b * 128:(b + 1) * 128],
                                rhs=v_sb[:, i, 2 * p + hh, :],
                                start=(i == 0 and b == 0),
                                stop=(i == 4 * j + 3 and b == 3))

                for i in range(4 * j + 4):
                    c0 = max(0, i * 128 - j * 512)
                    sc = psc.tile([128, 2, 512], F32, name="sc")
                    for hh in range(2):
                        nc.tensor.matmul(
                            sc[:, hh, c0:],
                            lhsT=kT_sb[p][hh * 64:(hh + 1) * 64,
                                          i * 128:(i + 1) * 128],
                            rhs=qT_sb[p][hh * 64:(hh + 1) * 64,
                                         j * 512 + c0:(j + 1) * 512],
                            start=True, stop=True,
                            tile_position=(hh * 64, 0))
                    pT = pTp.tile([128, 2, 512], BF16, name="pT")
                    nc.scalar.activation(pT[:, :, c0:], sc[:, :, c0:],
                                         AF.Exp, scale=0.125)
                    if i * 128 >= j * 512:  # diagonal block: causal mask
                        nc.vector.tensor_tensor(
                            pT[:, :, c0:c0 + 128],
                            pT[:, :, c0:c0 + 128],
                            tri[:, None, :].broadcast_to([128, 2, 128]),
                            op=OP.mult)
                    pv_pend.append((i, pT, c0 // 128))
                    if len(pv_pend) > LAG:
                        emit_pv(*pv_pend.popleft())
                    it += 1
                    # pace against iters+8 so a few thunks remain to cover
                    # the PV flushes at the chunk's end
                    target = (it * quota) // (iters + 8)
                    while drained < target and fillers:
                        fillers.popleft()()
                        drained += 1
                while pv_pend:
                    emit_pv(*pv_pend.popleft())
                    # keep PE fed while the tail exps drain on ACT
                    if fillers and drained < quota:
                        fillers.popleft()()
                        drained += 1

                # normalize: rowsums sit at col 64 of each block, per
                # partition -> cheap reciprocal + broadcast multiply
                a_tmp = atm.tile([128, 4, 128], BF16, name="a_tmp")
                rc = rcp.tile([128, 2, 4, 1], F32, name="rc")
                nc.vector.reciprocal(rc[:], at_ps[:, :, :, DH:DH + 1])
                nc.vector.tensor_tensor(
                    a_tmp[:].rearrange("p b (hh d) -> p hh b d", hh=2),
                    at_ps[:, :, :, 0:DH],
                    rc[:].broadcast_to([128, 2, 4, DH]), op=OP.mult)
                # back to attnT layout for the projection via xbar DMA
                nc.sync.dma_start_transpose(
                    aT_sb[:, p, j * 512:(j + 1) * 512].rearrange(
                        "p (b s) -> p b s", b=4),
                    a_tmp[:].rearrange("p b s -> p (b s)"))

            while fillers:     # B(j+1) must be emitted before C(j+1)
                fillers.popleft()()
            if j < NJ - 1:
                proj_deferred.extend(build_proj(j))

        for t in build_proj(NJ - 1, tail=True):
            t()


_NC = None


def _get_module():
    global _NC
    if _NC is None:
        _NC = build_module()
    return _NC


def make_in_maps(hidden_states, c_attn_w, c_attn_b, c_proj_w):
    in_maps = []
    for c in range(NCORES):
        b, g = c // 2, c % 2
        cols = slice(g * GD, (g + 1) * GD)
        bq = np.ascontiguousarray(
            c_attn_b[g * GD:(g + 1) * GD].reshape(MT, 128).T)
        bk = np.ascontiguousarray(
            c_attn_b[D + g * GD:D + (g + 1) * GD].reshape(MT, 128).T)
        in_maps.append({
            "xt": np.ascontiguousarray(hidden_states[b].T),
            "wq": np.ascontiguousarray(c_attn_w[:, cols]),
            "wk": np.ascontiguousarray(c_attn_w[:, D + g * GD:D + (g + 1) * GD]),
            "wv": np.ascontiguousarray(
                c_attn_w[:, 2 * D + g * GD:2 * D + (g + 1) * GD]),
            "wp": np.ascontiguousarray(c_proj_w[g * GD:(g + 1) * GD, :]),
            "bqk": np.concatenate([bq, bk], axis=1).astype(np.float32),
        })
    return in_maps


def kernel(hidden_states, c_attn_w, c_attn_b, c_proj_w, c_proj_b, _trace=False):
    hidden_states = np.asarray(hidden_states, dtype=np.float32)
    c_attn_w = np.asarray(c_attn_w, dtype=np.float32)
    c_attn_b = np.asarray(c_attn_b, dtype=np.float32)
    c_proj_w = np.asarray(c_proj_w, dtype=np.float32)
    c_proj_b = np.asarray(c_proj_b, dtype=np.float32)

    nc = _get_module()
    in_maps = make_in_maps(hidden_states, c_attn_w, c_attn_b, c_proj_w)
    res = run_bass_kernel_spmd(nc, in_maps, list(range(NCORES)), trace=_trace)

    # v-bias is folded here: attn rows sum to 1, so +b_v passes through the
    # attention average and lands as b_v @ c_proj_w on the output.
    bias_eff = c_proj_b + c_attn_b[2 * D:3 * D] @ c_proj_w
    outp = np.empty((B, S, D), dtype=np.float32)
    for b in range(B):
        outp[b] = (res.results[2 * b]["out"] + res.results[2 * b + 1]["out"]
                   + bias_eff[None, :])
    if _trace:
        return outp, res
    return outp
